# revision 15
# baseline (speedup 1.0000x reference)
"""Multi-head attention (B=8, S=1024, E=768, H=12, D=64) on 8 TRN2 NeuronCores.

Sharding: data-parallel over batch. Core i computes batch element i end to end;
weights are replicated. No collectives.

Per-core dataflow (all matmuls run as float32r, 1 cycle/row on the PE):
  1. x [S,E] -> PE-transpose -> xT [E,S] (f32r)
  2. qkvT = w_qkv.T @ xT for the Q,K blocks -> QT/KT [768,S] (cast bf16)
     V = xT.T @ w_qkv[:,2E:] -> V_pad [S, H*(D+1)] with a ones column per head
  3. per head pair (packed K=64 matmuls via tile_position):
     S^T[k,q] = K^T.T @ Q^T; exp on ACT (scale=1/8); PV: out^T[d,q] (+denom row)
     normalize via DVE reciprocal + DRAM-bounce partition-broadcast + multiply
  4. y = attnT.T @ w_out + b_out
"""

import numpy as np

import concourse.bass as bass
import concourse.bacc as bacc
import concourse.tile as tile
from concourse import mybir
from concourse.bass_utils import run_bass_kernel_spmd
from concourse.bass_interp import get_hw_module
from concourse.masks import make_identity

F32 = mybir.dt.float32
F32R = mybir.dt.float32r
BF16 = mybir.dt.bfloat16

B, S, E = 8, 1024, 768
H, D = 12, 64
F = 3 * E                  # 2304
NCORES = 8
NPAIR = H // 2             # 6 head pairs
NKC = S // 128             # 8 key chunks
NST = S // 128             # 8 sequence tiles
NE = E // 128              # 6 embedding chunks
DP = D + 1                 # 65: head dim + ones column

# dtype of exp(S^T) and V_pad (must match for the PV matmul)
PV_DT = F32R
QK_DT = BF16


def _build():
    nc = bacc.Bacc("TRN2", target_bir_lowering=False, debug=False,
                   num_devices=NCORES)

    x_d = nc.dram_tensor("x", [S, E], F32, kind="ExternalInput").ap()
    wqkv_d = nc.dram_tensor("w_qkv", [E, F], F32, kind="ExternalInput").ap()
    wout_d = nc.dram_tensor("w_out", [E, E], F32, kind="ExternalInput").ap()
    bout_d = nc.dram_tensor("b_out", [E], F32, kind="ExternalInput").ap()
    y_d = nc.dram_tensor("y", [S, E], F32, kind="ExternalOutput").ap()

    with tile.TileContext(nc) as tc:
        _emit(nc, tc, x_d, wqkv_d, wout_d, bout_d, y_d)

    nc.compile()
    nc.m = get_hw_module(nc.m)
    return nc


def _emit(nc, tc, x_d, wqkv_d, wout_d, bout_d, y_d):
    from contextlib import ExitStack
    ctx = ExitStack()
    with ctx:
        singles = ctx.enter_context(tc.tile_pool(name="singles", bufs=1))
        # persists across qkv + attention (+ proj for attnT)
        sb = ctx.enter_context(tc.tile_pool(name="sb", bufs=1))

        # ---- constants ----
        identity = singles.tile([128, 128], F32)
        make_identity(nc, identity)
        bias_bc = singles.tile([128, E], F32)
        nc.sync.dma_start(
            out=bias_bc,
            in_=bass.AP(tensor=bout_d.tensor, offset=bout_d.offset,
                        ap=[[0, 128]] + list(bout_d.ap)))
        ones_row = singles.tile([1, 64], F32R)
        nc.gpsimd.memset(ones_row.bitcast(mybir.dt.uint32), 0x3F800000)


        qkv_pool = tc.alloc_tile_pool(name="qkv_pool", bufs=1)

        # ---- weights ----
        wq = []
        for ei in range(NE):
            w_t = qkv_pool.tile([128, F], F32R, name=f"wqkv_{ei}")
            nc.sync.dma_start(out=w_t,
                              in_=wqkv_d[ei * 128:(ei + 1) * 128, :].bitcast(F32R))
            wq.append(w_t)

        # ---- load x and transpose to xT [E, S] ----
        x_sb = []
        for st in range(NST):
            x_t = qkv_pool.tile([128, E], F32, name=f"x_{st}")
            nc.sync.dma_start(out=x_t, in_=x_d[st * 128:(st + 1) * 128, :])
            x_sb.append(x_t)

        with tc.tile_pool(name="psA", bufs=4, space="PSUM") as psA:
            xT = []
            for ei in range(NE):
                ps_xt = psA.tile([128, S], F32, tag="mm", name=f"psxt_{ei}")
                for st in range(NST):
                    nc.tensor.transpose(
                        ps_xt[:, st * 128:(st + 1) * 128],
                        x_sb[st][:, ei * 128:(ei + 1) * 128],
                        identity)
                xt_t = qkv_pool.tile([128, S], F32R, name=f"xT_{ei}")
                nc.vector.tensor_copy(xt_t, ps_xt)
                xT.append(xt_t)

            # ---- V projection -> V_pad [S, H*DP] with ones cols ----
            v_pad = []
            for st in range(NST):
                ps_v = psA.tile([128, S], F32, tag="mm", name=f"psv_{st}")
                for (c0, cn) in ((0, 512), (512, 256)):
                    for ei in range(NE):
                        nc.tensor.matmul(
                            ps_v[:, c0:c0 + cn],
                            xT[ei][:, st * 128:(st + 1) * 128],
                            wq[ei][:, 2 * E + c0:2 * E + c0 + cn],
                            start=(ei == 0), stop=(ei == NE - 1))
                vp_t = sb.tile([128, H * DP], PV_DT, name=f"vpad_{st}")
                vp3 = vp_t.rearrange("p (h c) -> p h c", c=DP)
                # 0x3F800000 == 1.0f; uint32 view because walrus rejects
                # f32r-dtype memsets
                nc.gpsimd.memset(vp_t.bitcast(mybir.dt.uint32), 0x3F800000)
                nc.vector.tensor_copy(
                    vp3[:, :, 0:D],
                    ps_v[:, 0:E].rearrange("p (h d) -> p h d", d=D))
                v_pad.append(vp_t)

            # ---- Q^T, K^T projections (cast to bf16) ----
            qkT = []          # 12 tiles: 0-5 = Q^T pairs, 6-11 = K^T pairs
            for ft in range(2 * NE):
                ps_q = psA.tile([128, S], F32, tag="mm", name=f"psq_{ft}")
                for sc in range(2):
                    for ei in range(NE):
                        nc.tensor.matmul(
                            ps_q[:, sc * 512:(sc + 1) * 512],
                            wq[ei][:, ft * 128:(ft + 1) * 128],
                            xT[ei][:, sc * 512:(sc + 1) * 512],
                            start=(ei == 0), stop=(ei == NE - 1))
                qk_t = sb.tile([128, S], QK_DT, name=f"qkT_{ft}")
                nc.vector.tensor_copy(qk_t, ps_q)
                qkT.append(qk_t)

        qkv_pool.release()
        expst_pool = ctx.enter_context(tc.tile_pool(name="expst", bufs=3))
        bcast_pool = ctx.enter_context(tc.tile_pool(name="bcast", bufs=2))
        rc_pool = ctx.enter_context(tc.tile_pool(name="rc", bufs=2))
        wpool = ctx.enter_context(tc.tile_pool(name="wpool", bufs=1))
        ypool = ctx.enter_context(tc.tile_pool(name="ypool", bufs=2))

        # ---- attention, one head pair at a time ----
        attnT = []
        with tc.tile_pool(name="psB", bufs=1, space="PSUM") as psB:
            for j in range(NPAIR):
                qT = qkT[j]
                kT = qkT[NE + j]
                ps_pv = [
                    psB.tile([DP, S], F32, tag="pv", bufs=2, name=f"pspv_{j}_{hh}")
                    for hh in range(2)]
                for kc in range(NKC):
                    ps_s = psB.tile([128, 2 * S], F32, tag="scores",
                                    name=f"pss_{j}_{kc}")
                    for qc in range(2):
                        for hh in range(2):
                            nc.tensor.matmul(
                                ps_s[:, hh * S + qc * 512: hh * S + (qc + 1) * 512],
                                kT[hh * 64:(hh + 1) * 64,
                                   kc * 128:(kc + 1) * 128],
                                qT[hh * 64:(hh + 1) * 64,
                                   qc * 512:(qc + 1) * 512],
                                start=True, stop=True,
                                tile_position=(hh * 64, 0))
                    expst = expst_pool.tile([128, 2 * S], PV_DT, tag="expst",
                                            name=f"expst_{j}_{kc}")
                    nc.scalar.activation(out=expst, in_=ps_s,
                                         func=mybir.ActivationFunctionType.Exp,
                                         scale=0.125)
                    for hh in range(2):
                        for qc in range(2):
                            nc.tensor.matmul(
                                ps_pv[hh][:, qc * 512:(qc + 1) * 512],
                                v_pad[kc][:, (2 * j + hh) * DP:
                                           (2 * j + hh + 1) * DP],
                                expst[:, hh * S + qc * 512: hh * S + (qc + 1) * 512],
                                start=(kc == 0), stop=(kc == NKC - 1))

                # softmax denominators -> reciprocal -> PE broadcast -> scale
                rc_t = rc_pool.tile([1, 2 * S], F32R, tag="rc", name=f"rc_{j}")
                with nc.allow_low_precision(reason="f32r recip storage"):
                    for hh in range(2):
                        nc.vector.reciprocal(rc_t[0:1, hh * S:(hh + 1) * S],
                                             ps_pv[hh][D:DP, :])
                bc_ps = psB.tile([64, 2 * S], F32, tag="scores",
                                 name=f"bcps_{j}")
                for hh in range(2):
                    for qc in range(2):
                        c0 = hh * S + qc * 512
                        nc.tensor.matmul(bc_ps[:, c0:c0 + 512], ones_row,
                                         rc_t[0:1, c0:c0 + 512],
                                         start=True, stop=True)
                bc_sb = bcast_pool.tile([64, 2 * S], F32, tag="bc",
                                        name=f"bc_{j}")
                nc.vector.tensor_copy(bc_sb, bc_ps)
                at_t = sb.tile([128, S], F32R, name=f"attnT_{j}")
                for hh in range(2):
                    nc.vector.tensor_mul(
                        at_t[hh * 64:(hh + 1) * 64, :],
                        ps_pv[hh][0:D, :],
                        bc_sb[:, hh * S:(hh + 1) * S])
                attnT.append(at_t)

        # ---- output projection ----
        wo = []
        for ei in range(NE):
            wo_t = wpool.tile([128, E], F32R, name=f"wout_{ei}")
            nc.sync.dma_start(out=wo_t,
                              in_=wout_d[ei * 128:(ei + 1) * 128, :].bitcast(F32R))
            wo.append(wo_t)

        with tc.tile_pool(name="psC", bufs=4, space="PSUM") as psC:
            for st in range(NST):
                ps_y = psC.tile([128, S], F32, tag="yproj", name=f"psy_{st}")
                for (c0, cn) in ((0, 512), (512, 256)):
                    for ej in range(NE):
                        nc.tensor.matmul(
                            ps_y[:, c0:c0 + cn],
                            attnT[ej][:, st * 128:(st + 1) * 128],
                            wo[ej][:, c0:c0 + cn],
                            start=(ej == 0), stop=(ej == NE - 1))
                y_t = ypool.tile([128, E], F32, tag="y", name=f"y_{st}")
                nc.vector.tensor_add(y_t, ps_y[:, 0:E], bias_bc)
                nc.sync.dma_start(out=y_d[st * 128:(st + 1) * 128, :], in_=y_t)


_NC_CACHE = None


def _get_nc():
    global _NC_CACHE
    if _NC_CACHE is None:
        _NC_CACHE = _build()
    return _NC_CACHE


def kernel(x, w_qkv, w_out, b_out, _trace=False, **_run_kwargs):
    """Full-input MHA: x [8,1024,768] f32 -> y [8,1024,768] f32."""
    nc = _get_nc()
    x = np.ascontiguousarray(np.asarray(x, dtype=np.float32))
    w_qkv = np.ascontiguousarray(np.asarray(w_qkv, dtype=np.float32))
    w_out = np.ascontiguousarray(np.asarray(w_out, dtype=np.float32))
    b_out = np.ascontiguousarray(np.asarray(b_out, dtype=np.float32))
    in_maps = [
        {"x": x[i], "w_qkv": w_qkv, "w_out": w_out, "b_out": b_out}
        for i in range(NCORES)
    ]
    res = run_bass_kernel_spmd(nc, in_maps, core_ids=list(range(NCORES)),
                               trace=_trace, **_run_kwargs)
    y = np.stack([res.results[i]["y"] for i in range(NCORES)], axis=0)
    if _trace:
        return y, res
    return y


# revision 19
# speedup vs baseline: 1.6874x; 1.6874x over previous
"""Multi-head attention (B=8, S=1024, E=768, H=12, D=64) on 8 TRN2 NeuronCores.

Sharding: data-parallel over batch. Core i computes batch element i end to end;
weights are replicated. No collectives.

Per-core dataflow (all matmuls run as float32r, 1 cycle/row on the PE):
  1. x [S,E] -> PE-transpose -> xT [E,S] (f32r)
  2. qkvT = w_qkv.T @ xT for the Q,K blocks -> QT/KT [768,S] (cast bf16)
     V = xT.T @ w_qkv[:,2E:] -> V_pad [S, H*(D+1)] with a ones column per head
  3. per head pair (packed K=64 matmuls via tile_position):
     S^T[k,q] = K^T.T @ Q^T; exp on ACT (scale=1/8); PV: out^T[d,q] (+denom row)
     normalize via DVE reciprocal + DRAM-bounce partition-broadcast + multiply
  4. y = attnT.T @ w_out + b_out
"""

import numpy as np

import concourse.bass as bass
import concourse.bacc as bacc
import concourse.tile as tile
from concourse import mybir
from concourse.bass_utils import run_bass_kernel_spmd
from concourse.bass_interp import get_hw_module
from concourse.masks import make_identity

F32 = mybir.dt.float32
F32R = mybir.dt.float32r
BF16 = mybir.dt.bfloat16

B, S, E = 8, 1024, 768
H, D = 12, 64
F = 3 * E                  # 2304
NCORES = 8
NPAIR = H // 2             # 6 head pairs
NKC = S // 128             # 8 key chunks
NST = S // 128             # 8 sequence tiles
NE = E // 128              # 6 embedding chunks
DP = D + 1                 # 65: head dim + ones column

# dtype of exp(S^T) and V_pad (must match for the PV matmul)
PV_DT = F32R
QK_DT = BF16


def _build():
    nc = bacc.Bacc("TRN2", target_bir_lowering=False, debug=False,
                   num_devices=NCORES)

    x_d = nc.dram_tensor("x", [S, E], F32, kind="ExternalInput").ap()
    wqkv_d = nc.dram_tensor("w_qkv", [E, F], F32, kind="ExternalInput").ap()
    wout_d = nc.dram_tensor("w_out", [E, E], F32, kind="ExternalInput").ap()
    bout_d = nc.dram_tensor("b_out", [E], F32, kind="ExternalInput").ap()
    y_d = nc.dram_tensor("y", [S, E], F32, kind="ExternalOutput").ap()

    with tile.TileContext(nc) as tc:
        _emit(nc, tc, x_d, wqkv_d, wout_d, bout_d, y_d)

    nc.compile()
    nc.m = get_hw_module(nc.m)
    return nc


def _emit(nc, tc, x_d, wqkv_d, wout_d, bout_d, y_d):
    from contextlib import ExitStack
    ctx = ExitStack()
    with ctx:
        singles = ctx.enter_context(tc.tile_pool(name="singles", bufs=1))
        # persists across qkv + attention (+ proj for attnT)
        sb = ctx.enter_context(tc.tile_pool(name="sb", bufs=1))

        # ---- constants ----
        identity = singles.tile([128, 128], F32)
        make_identity(nc, identity)
        bias_bc = singles.tile([128, E], F32)
        nc.sync.dma_start(
            out=bias_bc,
            in_=bass.AP(tensor=bout_d.tensor, offset=bout_d.offset,
                        ap=[[0, 128]] + list(bout_d.ap)))
        ones_row = singles.tile([1, 64], F32R)
        nc.gpsimd.memset(ones_row.bitcast(mybir.dt.uint32), 0x3F800000)


        qkv_pool = tc.alloc_tile_pool(name="qkv_pool", bufs=1)

        # ---- weights ----
        wq = []
        for ei in range(NE):
            w_t = qkv_pool.tile([128, F], F32R, name=f"wqkv_{ei}")
            nc.sync.dma_start(out=w_t,
                              in_=wqkv_d[ei * 128:(ei + 1) * 128, :].bitcast(F32R))
            wq.append(w_t)

        # ---- load x and transpose to xT [E, S] ----
        x_sb = []
        for st in range(NST):
            x_t = qkv_pool.tile([128, E], F32, name=f"x_{st}")
            nc.sync.dma_start(out=x_t, in_=x_d[st * 128:(st + 1) * 128, :])
            x_sb.append(x_t)

        with tc.tile_pool(name="psA", bufs=4, space="PSUM") as psA:
            xT = []
            for ei in range(NE):
                ps_xt = psA.tile([128, S], F32, tag="mm", name=f"psxt_{ei}")
                for st in range(NST):
                    nc.tensor.transpose(
                        ps_xt[:, st * 128:(st + 1) * 128],
                        x_sb[st][:, ei * 128:(ei + 1) * 128],
                        identity)
                xt_t = qkv_pool.tile([128, S], F32R, name=f"xT_{ei}")
                nc.vector.tensor_copy(xt_t, ps_xt)
                xT.append(xt_t)

            # ---- V projection -> V_pad [S, H*DP] with ones cols ----
            v_pad = []
            for st in range(NST):
                ps_v = psA.tile([128, S], F32, tag="mm", name=f"psv_{st}")
                for ei in range(NE):
                    for (c0, cn) in ((0, 512), (512, 256)):
                        nc.tensor.matmul(
                            ps_v[:, c0:c0 + cn],
                            xT[ei][:, st * 128:(st + 1) * 128],
                            wq[ei][:, 2 * E + c0:2 * E + c0 + cn],
                            start=(ei == 0), stop=(ei == NE - 1))
                vp_t = sb.tile([128, H * DP], PV_DT, name=f"vpad_{st}")
                vp3 = vp_t.rearrange("p (h c) -> p h c", c=DP)
                # 0x3F800000 == 1.0f; uint32 view because walrus rejects
                # f32r-dtype memsets
                nc.gpsimd.memset(vp_t.bitcast(mybir.dt.uint32), 0x3F800000)
                nc.vector.tensor_copy(
                    vp3[:, :, 0:D],
                    ps_v[:, 0:E].rearrange("p (h d) -> p h d", d=D))
                v_pad.append(vp_t)

            # ---- Q^T, K^T projections (cast to bf16) ----
            qkT = []          # 12 tiles: 0-5 = Q^T pairs, 6-11 = K^T pairs
            for ft in range(2 * NE):
                ps_q = psA.tile([128, S], F32, tag="mm", name=f"psq_{ft}")
                for ei in range(NE):
                    for sc in range(2):
                        nc.tensor.matmul(
                            ps_q[:, sc * 512:(sc + 1) * 512],
                            wq[ei][:, ft * 128:(ft + 1) * 128],
                            xT[ei][:, sc * 512:(sc + 1) * 512],
                            start=(ei == 0), stop=(ei == NE - 1))
                qk_t = sb.tile([128, S], QK_DT, name=f"qkT_{ft}")
                nc.vector.tensor_copy(qk_t, ps_q)
                qkT.append(qk_t)

        qkv_pool.release()
        expst_pool = ctx.enter_context(tc.tile_pool(name="expst", bufs=3))
        bcast_pool = ctx.enter_context(tc.tile_pool(name="bcast", bufs=2))
        rc_pool = ctx.enter_context(tc.tile_pool(name="rc", bufs=2))
        wpool = ctx.enter_context(tc.tile_pool(name="wpool", bufs=1))
        ypool = ctx.enter_context(tc.tile_pool(name="ypool", bufs=2))

        # ---- attention, one head pair at a time ----
        attnT = []
        with tc.tile_pool(name="psB", bufs=1, space="PSUM") as psB:
            for j in range(NPAIR):
                qT = qkT[j]
                kT = qkT[NE + j]
                ps_pv = [
                    psB.tile([DP, S], F32, tag="pv", bufs=2, name=f"pspv_{j}_{hh}")
                    for hh in range(2)]
                for kc in range(NKC):
                    # per-head scores slots (bufs=2) so exp(A) overlaps
                    # scores(B) and the next kc's scores
                    ps_s = [psB.tile([128, S], F32, tag="scores", bufs=2,
                                     name=f"pss_{j}_{kc}_{hh}")
                            for hh in range(2)]
                    expst = expst_pool.tile([128, 2 * S], PV_DT, tag="expst",
                                            name=f"expst_{j}_{kc}")
                    for hh in range(2):
                        for qc in range(2):
                            nc.tensor.matmul(
                                ps_s[hh][:, qc * 512:(qc + 1) * 512],
                                kT[hh * 64:(hh + 1) * 64,
                                   kc * 128:(kc + 1) * 128],
                                qT[hh * 64:(hh + 1) * 64,
                                   qc * 512:(qc + 1) * 512],
                                start=True, stop=True,
                                tile_position=(hh * 64, 0))
                        nc.scalar.activation(
                            out=expst[:, hh * S:(hh + 1) * S], in_=ps_s[hh],
                            func=mybir.ActivationFunctionType.Exp,
                            scale=0.125)
                    for hh in range(2):
                        for qc in range(2):
                            nc.tensor.matmul(
                                ps_pv[hh][:, qc * 512:(qc + 1) * 512],
                                v_pad[kc][:, (2 * j + hh) * DP:
                                           (2 * j + hh + 1) * DP],
                                expst[:, hh * S + qc * 512: hh * S + (qc + 1) * 512],
                                start=(kc == 0), stop=(kc == NKC - 1))

                # denominators (ACT copy to f32r) -> PE broadcast -> fast
                # reciprocal on 64 lanes -> scale
                rc_t = rc_pool.tile([1, 2 * S], F32R, tag="rc", name=f"rc_{j}")
                for hh in range(2):
                    nc.scalar.copy(rc_t[0:1, hh * S:(hh + 1) * S],
                                   ps_pv[hh][D:DP, :])
                bc_sb = bcast_pool.tile([64, 2 * S], F32, tag="bc",
                                        name=f"bc_{j}")
                for hh in range(2):
                    bc_ps = psB.tile([64, S], F32, tag="scores", bufs=2,
                                     name=f"bcps_{j}_{hh}")
                    for qc in range(2):
                        c0 = hh * S + qc * 512
                        nc.tensor.matmul(bc_ps[:, qc * 512:(qc + 1) * 512],
                                         ones_row, rc_t[0:1, c0:c0 + 512],
                                         start=True, stop=True)
                    nc.vector.reciprocal_approx_fast(
                        out=bc_sb[:, hh * S:(hh + 1) * S], in_=bc_ps)
                at_t = sb.tile([128, S], F32R, name=f"attnT_{j}")
                for hh in range(2):
                    nc.vector.tensor_mul(
                        at_t[hh * 64:(hh + 1) * 64, :],
                        ps_pv[hh][0:D, :],
                        bc_sb[:, hh * S:(hh + 1) * S])
                attnT.append(at_t)

        # ---- output projection ----
        wo = []
        for ei in range(NE):
            wo_t = wpool.tile([128, E], F32R, name=f"wout_{ei}")
            nc.sync.dma_start(out=wo_t,
                              in_=wout_d[ei * 128:(ei + 1) * 128, :].bitcast(F32R))
            wo.append(wo_t)

        with tc.tile_pool(name="psC", bufs=4, space="PSUM") as psC:
            for st in range(NST):
                ps_y = psC.tile([128, S], F32, tag="yproj", name=f"psy_{st}")
                for (c0, cn) in ((0, 512), (512, 256)):
                    for ej in range(NE):
                        nc.tensor.matmul(
                            ps_y[:, c0:c0 + cn],
                            attnT[ej][:, st * 128:(st + 1) * 128],
                            wo[ej][:, c0:c0 + cn],
                            start=(ej == 0), stop=(ej == NE - 1))
                y_t = ypool.tile([128, E], F32, tag="y", name=f"y_{st}")
                nc.vector.tensor_add(y_t, ps_y[:, 0:E], bias_bc)
                nc.sync.dma_start(out=y_d[st * 128:(st + 1) * 128, :], in_=y_t)


_NC_CACHE = None


def _get_nc():
    global _NC_CACHE
    if _NC_CACHE is None:
        _NC_CACHE = _build()
    return _NC_CACHE


def kernel(x, w_qkv, w_out, b_out, _trace=False, **_run_kwargs):
    """Full-input MHA: x [8,1024,768] f32 -> y [8,1024,768] f32."""
    nc = _get_nc()
    x = np.ascontiguousarray(np.asarray(x, dtype=np.float32))
    w_qkv = np.ascontiguousarray(np.asarray(w_qkv, dtype=np.float32))
    w_out = np.ascontiguousarray(np.asarray(w_out, dtype=np.float32))
    b_out = np.ascontiguousarray(np.asarray(b_out, dtype=np.float32))
    in_maps = [
        {"x": x[i], "w_qkv": w_qkv, "w_out": w_out, "b_out": b_out}
        for i in range(NCORES)
    ]
    res = run_bass_kernel_spmd(nc, in_maps, core_ids=list(range(NCORES)),
                               trace=_trace, **_run_kwargs)
    y = np.stack([res.results[i]["y"] for i in range(NCORES)], axis=0)
    if _trace:
        return y, res
    return y
